# revision 28
# baseline (speedup 1.0000x reference)
"""Multi-head attention Trainium2 kernel, 8-core batch+head sharded.

Sharding: cores 0-3 -> batch 0, cores 4-7 -> batch 1; each core computes 4
heads. Host compacts queries by q_mask and keys by v_mask (masked softmax
over the kept key subset equals the reference's additive-mask softmax),
transposes/packs inputs, and sums the 4 per-core partial output projections
per batch (the row-sharded-Wo "all-reduce"), adds bo, scatters rows back.

Self-contained: hardcodes B=2,S=2048,D=1024,H=16,HS=64,OUT=1024.
"""
import sys, types

sys.path.insert(0, '/opt/trn_rl_repo')

# ---- NTFF profile hook (image's antenv lacks axon_hooks) ----
if "antenv.axon_hooks" not in sys.modules:
    _hook_mod = types.ModuleType("antenv.axon_hooks")
    _hook_mod._hook = None
    def _set_hook(h, _m=_hook_mod):
        _m._hook = h
    def _get_hook(_m=_hook_mod):
        return _m._hook
    _hook_mod.set_axon_ntff_profile_hook = _set_hook
    _hook_mod.get_axon_ntff_profile_hook = _get_hook
    sys.modules["antenv.axon_hooks"] = _hook_mod
    try:
        from trn_agent_boot.trn_boot import _ntff_profile_via_ctypes
        _set_hook(_ntff_profile_via_ctypes('/opt/axon/libaxon_pjrt.so'))
    except Exception:
        pass

import numpy as np
import ml_dtypes
import concourse.bass as bass
import concourse.tile as tile
import concourse.mybir as mybir
from concourse import bass_utils, bacc

B, S, D, H, HS, OUT = 2, 2048, 1024, 16, 64, 1024
HPC = 4          # heads per core
NCORES = 8
DT = D // 128    # 8 d-tiles
F32 = mybir.dt.float32
F32R = mybir.dt.float32r
BF16 = mybir.dt.bfloat16
F16 = mybir.dt.float16
DT_MM = F32R     # on-device tiles for QK/outproj operands
DT_IN = F16      # DMA'd input dtype (half the bytes, 2^-11 rounding)
DT_AV = F16      # AV/Z operand dtype (fp32r cannot col-tile)
SCALE = float(1.0 / np.sqrt(HS))
KPAD_BIAS = -1e5  # exp underflows to exactly 0.0
BLK = 1024       # sq/sk block width (PSUM-bank driven)


def _chunks(total, maxc=512):
    """512s + remainder: chunk offsets stay PSUM-bank-aligned."""
    out = [maxc] * (total // maxc)
    if total % maxc:
        out.append(total % maxc)
    return out


def _blocks(total, w=BLK):
    return [(i * w, min(w, total - i * w)) for i in range((total + w - 1) // w)]


def _bblocks(total):
    """512-wide blocks + remainder; matmul PSUM dsts stay bank-aligned
    because S tiles are allocated at the padded [128, 2, 512] shape."""
    out = [512] * (total // 512)
    if total % 512:
        out.append(total % 512)
    return out


def build_kernel(SQP, SKP):
    """One SPMD Bass program. SQP/SKP: padded (mult of 128) query/key counts."""
    SKT = SKP // 128
    nc = bacc.Bacc("TRN2", target_bir_lowering=False, debug=False,
                   num_devices=NCORES)

    xq_d = nc.dram_tensor('xq', [DT, 128, SQP], DT_IN, kind='ExternalInput').ap()
    xk_d = nc.dram_tensor('xk', [DT, 128, SKP], DT_IN, kind='ExternalInput').ap()
    xv_d = nc.dram_tensor('xv', [DT, 128, SKP], DT_IN, kind='ExternalInput').ap()
    wq_d = nc.dram_tensor('wq', [DT, 128, 256], DT_IN, kind='ExternalInput').ap()
    wk_d = nc.dram_tensor('wk', [DT, 128, 256], DT_IN, kind='ExternalInput').ap()
    wv_d = nc.dram_tensor('wv', [DT, 128, 256], DT_IN, kind='ExternalInput').ap()
    wo_d = nc.dram_tensor('wo', [2, 128, OUT], F32, kind='ExternalInput').ap()
    qkb_d = nc.dram_tensor('qkb', [128, 4], F32, kind='ExternalInput').ap()
    vb_d = nc.dram_tensor('vb', [1, 256], F32, kind='ExternalInput').ap()
    kbias_d = nc.dram_tensor('kbias', [128, SKT], F32, kind='ExternalInput').ap()
    outp = nc.dram_tensor('outp', [SQP, OUT], F16, kind='ExternalOutput').ap()

    with tile.TileContext(nc) as tc, \
         nc.allow_low_precision(reason="float32r tiles are fp32-width"):
        with tc.tile_pool(name="const", bufs=1) as constp, \
             tc.tile_pool(name="xin", bufs=6) as xin, \
             tc.tile_pool(name="persist", bufs=1) as persist, \
             tc.tile_pool(name="etile", bufs=8) as etile, \
             tc.tile_pool(name="work", bufs=2) as work:

            # ---- constants ----
            wq_sb = constp.tile([128, DT, 256], DT_IN)
            wk_sb = constp.tile([128, DT, 256], DT_IN)
            wv_sb = constp.tile([128, DT, 256], DT_IN)
            wo_sb = constp.tile([128, 2, OUT], DT_MM)
            qkb_sb = constp.tile([128, 4], F32)
            vb_bc = constp.tile([128, 256], F32)
            kbias_sb = constp.tile([128, SKT], F32)
            ones_f = constp.tile([128, 64], F32)
            ones_h = constp.tile([128, 64], DT_AV)
            ones_r = constp.tile([128, 64], DT_MM)
            for t in range(DT):
                nc.sync.dma_start(out=wq_sb[:, t, :], in_=wq_d[t])
            nc.sync.dma_start(out=qkb_sb, in_=qkb_d)
            nc.sync.dma_start(out=vb_bc, in_=bass.AP(
                tensor=vb_d.tensor, offset=vb_d.offset,
                ap=[[0, 128], vb_d.ap[1]]))
            nc.sync.dma_start(out=kbias_sb, in_=kbias_d)
            nc.vector.memset(ones_f, 1.0)
            nc.vector.tensor_copy(ones_h, ones_f)
            nc.vector.tensor_copy(ones_r, ones_f)

            # ---- persistent activations ----
            qt_sb = persist.tile([128, 2, SQP], DT_MM)   # [:, pair, :]: Q^T 2 heads stacked
            kt_sb = persist.tile([128, 2, SKP], DT_MM)
            v_sb = persist.tile([128, SKT, 256], DT_AV)  # V natural, 4 heads
            ot_sb = persist.tile([128, 2, SQP], DT_MM)   # normalized O^T (outproj lhsT)
            z_sb = persist.tile([128, SQP], F32)         # Z at partition rows 32h
            zinv_sb = persist.tile([128, SQP], DT_MM)
            nc.vector.memset(z_sb, 0.0)

            # ---- stage A: Q/K projections (transposed out, col-packed pairs) ----
            with tc.tile_pool(name="psA", bufs=1, space="PSUM") as psA, \
                 tc.tile_pool(name="psV", bufs=2, space="PSUM") as psV:
                for wtag, xd, w_sb, pt_sb, tot, bcol0 in (
                        ("q", xq_d, wq_sb, qt_sb, SQP, 0),
                        ("k", xk_d, wk_sb, kt_sb, SKP, 2)):
                    if wtag == "k":
                        for t in range(DT):
                            nc.sync.dma_start(out=wk_sb[:, t, :], in_=wk_d[t])
                    for b0, blen in _blocks(tot, 1280):
                        pp = [psA.tile([128, blen], F32, tag=f"proj{p}", name=f"pp{p}")
                              for p in range(2)]
                        for t in range(DT):
                            xt = xin.tile([128, blen], DT_IN, tag="x")
                            nc.sync.dma_start(
                                out=xt, in_=xd[t, :, b0:b0 + blen])
                            off = 0
                            for chl in _chunks(blen):
                                for pair in range(2):
                                    nc.tensor.matmul(
                                        pp[pair][:, off:off + chl],
                                        w_sb[:, t, pair * 128:(pair + 1) * 128],
                                        xt[:, off:off + chl],
                                        start=(t == 0), stop=(t == DT - 1))
                                off += chl
                        for pair in range(2):
                            nc.vector.tensor_scalar_add(
                                pt_sb[:, pair, b0:b0 + blen], pp[pair],
                                qkb_sb[:, bcol0 + pair: bcol0 + pair + 1])

                # ---- stage A: V projection (natural layout, all 4 heads) ----
                for t in range(DT):
                    nc.sync.dma_start(out=wv_sb[:, t, :], in_=wv_d[t])
                for t in range(2):
                    nc.sync.dma_start(out=wo_sb[:, t, :], in_=wo_d.bitcast(DT_MM)[t])
                xv_sb = persist.tile([128, DT, SKP], DT_AV)
                for t in range(DT):
                    nc.sync.dma_start(out=xv_sb[:, t, :], in_=xv_d[t])
                for skt in range(SKT):
                    pv = psV.tile([128, 256], F32, tag="pv")
                    for t in range(DT):
                        nc.tensor.matmul(
                            pv, xv_sb[:, t, skt * 128:(skt + 1) * 128],
                            wv_sb[:, t, :],
                            start=(t == 0), stop=(t == DT - 1))
                    nc.vector.tensor_add(v_sb[:, skt, :], pv, vb_bc)

            # ---- stages B+C per sq block (2-head-fused S tiles, bank-aligned) ----
            with tc.tile_pool(name="psS", bufs=3, space="PSUM") as psS, \
                 tc.tile_pool(name="psO", bufs=1, space="PSUM") as psO:
                bq0 = 0
                for bqlen in _bblocks(SQP):
                    opsum = [psO.tile([128, bqlen], F32, tag=f"opsum{p}", name=f"op{p}")
                             for p in range(2)]
                    def emit_avz(skt, e2s):
                        # AV: V stationary, col-packed 2 heads per pair
                        for pair in range(2):
                            for hh in range(2):
                                h = pair * 2 + hh
                                nc.tensor.matmul(
                                    opsum[pair][hh * 64:(hh + 1) * 64, :],
                                    v_sb[:, skt, h * 64:(h + 1) * 64],
                                    e2s[pair][:, hh, :],
                                    start=(skt == 0), stop=(skt == SKT - 1))
                        # Z: 4-way col-tiled ones-matmuls, accumulated in SBUF
                        zp = psS.tile([128, bqlen], F32, tag="s2", name="zp")
                        for h in range(HPC):
                            pair, hh = divmod(h, 2)
                            nc.tensor.matmul(
                                zp[32 * h:32 * h + 1, :],
                                ones_h[:, 0:1], e2s[pair][:, hh, :],
                                start=True, stop=True,
                                tile_position=(0, 32 * h))
                        nc.vector.tensor_add(
                            z_sb[:, bq0:bq0 + bqlen],
                            z_sb[:, bq0:bq0 + bqlen], zp)

                    prev = None
                    for skt in range(SKT):
                        e2s = []
                        for pair in range(2):
                            # QK: 2 heads into one S tile, row-packed
                            st2 = psS.tile([128, 2, 512], F32, tag="s2",
                                           name=f"st{pair}")
                            for hh in range(2):
                                nc.tensor.matmul(
                                    st2[:, hh, :bqlen],
                                    kt_sb[hh * 64:(hh + 1) * 64, pair,
                                          skt * 128:(skt + 1) * 128],
                                    qt_sb[hh * 64:(hh + 1) * 64, pair,
                                          bq0:bq0 + bqlen],
                                    start=True, stop=True)
                            # exp of both heads in one ScalarE op (mask+scale fused)
                            e2 = etile.tile([128, 2, bqlen], DT_AV, tag="e",
                                            name=f"e{pair}")
                            nc.scalar.activation(
                                e2, st2[:, :, :bqlen],
                                mybir.ActivationFunctionType.Exp,
                                bias=kbias_sb[:, skt:skt + 1], scale=SCALE)
                            e2s.append(e2)
                        if prev is not None:
                            emit_avz(prev[0], prev[1])
                        prev = (skt, e2s)
                    emit_avz(prev[0], prev[1])
                    # ---- stage C: normalize + output projection ----
                    # one full-width reciprocal (a [1,N] op uses 1 of 128 DVE
                    # lanes); rows other than 32h are 1/0=inf, never read
                    nc.vector.reciprocal(zinv_sb[:, bq0:bq0 + bqlen],
                                         z_sb[:, bq0:bq0 + bqlen])
                    for pair in range(2):
                        for hh in range(2):
                            h = pair * 2 + hh
                            zbc = psS.tile([64, bqlen], F32, tag="s2",
                                           name=f"zbc{h}")
                            nc.tensor.matmul(
                                zbc,
                                ones_r[32 * h:32 * h + 1, 0:64],
                                zinv_sb[32 * h:32 * h + 1, bq0:bq0 + bqlen],
                                start=True, stop=True,
                                tile_position=(32 * h, 0))
                            zbc_sb = work.tile([64, bqlen], F32, tag="zbc",
                                               name=f"zbcs{h}")
                            nc.vector.tensor_copy(zbc_sb, zbc)
                            nc.vector.tensor_mul(
                                ot_sb[hh * 64:(hh + 1) * 64, pair,
                                      bq0:bq0 + bqlen],
                                opsum[pair][hh * 64:(hh + 1) * 64, :],
                                zbc_sb)
                    for sqt in range(bqlen // 128):
                        po = psS.tile([128, OUT], F32, tag="s2", name="po")
                        for kt in range(2):
                            for ch in range(2):
                                nc.tensor.matmul(
                                    po[:, ch * 512:(ch + 1) * 512],
                                    ot_sb[:, kt, bq0 + sqt * 128:
                                          bq0 + (sqt + 1) * 128],
                                    wo_sb[:, kt, ch * 512:(ch + 1) * 512],
                                    start=(kt == 0), stop=(kt == 1))
                        ob = work.tile([128, OUT], F16, tag="ob")
                        if sqt % 2 == 0:
                            nc.vector.tensor_copy(ob, po)
                        else:
                            nc.scalar.copy(ob, po)
                        nc.sync.dma_start(
                            out=outp[bq0 + sqt * 128: bq0 + (sqt + 1) * 128, :],
                            in_=ob)
                    bq0 += bqlen

    nc.compile()
    return nc


_NC_CACHE = {}


def _get_kernel(SQP, SKP):
    key = (SQP, SKP)
    if key not in _NC_CACHE:
        _NC_CACHE[key] = build_kernel(SQP, SKP)
    return _NC_CACHE[key]


def _ref_numpy(q, k, v, Wq, bq, Wk, bk, Wv, bv, Wo, bo, qm, vm):
    """Exact-reference fallback for degenerate masks (all-zero v_mask)."""
    qp = (q @ Wq + bq).reshape(S, H, HS)
    kp = (k @ Wk + bk).reshape(S, H, HS)
    vp = (v @ Wv + bv).reshape(S, H, HS)
    a = np.einsum('qhd,khd->hqk', qp, kp) / np.sqrt(HS)
    a = a - (1.0 - vm[None, None, :]) * 1e12
    a = a - a.max(-1, keepdims=True)
    e = np.exp(a)
    p = e / e.sum(-1, keepdims=True)
    o = np.einsum('hqk,khd->qhd', p, vp).reshape(S, H * HS)
    return (o @ Wo + bo) * qm[:, None]


def run(query, key, value, Wq, bq, Wk, bk, Wv, bv, Wo, bo, q_mask, v_mask,
        trace=False):
    query = np.asarray(query, np.float32)
    key = np.asarray(key, np.float32)
    value = np.asarray(value, np.float32)
    Wq, bq = np.asarray(Wq, np.float32), np.asarray(bq, np.float32)
    Wk, bk = np.asarray(Wk, np.float32), np.asarray(bk, np.float32)
    Wv, bv = np.asarray(Wv, np.float32), np.asarray(bv, np.float32)
    Wo, bo = np.asarray(Wo, np.float32), np.asarray(bo, np.float32)
    q_mask = np.asarray(q_mask)
    v_mask = np.asarray(v_mask)

    qidx = [np.nonzero(q_mask[b])[0] for b in range(B)]
    kidx = [np.nonzero(v_mask[b])[0] for b in range(B)]
    host_fallback = [len(kidx[b]) == 0 for b in range(B)]

    nq = max([128] + [len(i) for b, i in enumerate(qidx) if not host_fallback[b]])
    nk = max([128] + [len(i) for b, i in enumerate(kidx) if not host_fallback[b]])
    SQP = ((nq + 127) // 128) * 128
    SKP = ((nk + 127) // 128) * 128
    SKT = SKP // 128

    nc = _get_kernel(SQP, SKP)

    in_maps = []
    for c in range(NCORES):
        b, hg = c // 4, c % 4
        hc = slice(hg * HPC * HS, (hg + 1) * HPC * HS)  # this core's 256 head cols
        xq = np.zeros((SQP, D), np.float32)
        xk = np.zeros((SKP, D), np.float32)
        xv = np.zeros((SKP, D), np.float32)
        if not host_fallback[b]:
            xq[:len(qidx[b])] = query[b][qidx[b]]
            xk[:len(kidx[b])] = key[b][kidx[b]]
            xv[:len(kidx[b])] = value[b][kidx[b]]
        qkb = np.stack([bq[hc][:128], bq[hc][128:],
                        bk[hc][:128], bk[hc][128:]], axis=1)
        nkb = len(kidx[b]) if not host_fallback[b] else 0
        kbias = np.where(np.arange(SKP) < nkb, 0.0, KPAD_BIAS).astype(np.float32)
        in_maps.append({
            'xq': np.ascontiguousarray(xq.T.reshape(DT, 128, SQP)).astype(np.float16),
            'xk': np.ascontiguousarray(xk.T.reshape(DT, 128, SKP)).astype(np.float16),
            'xv': np.ascontiguousarray(xv.T.reshape(DT, 128, SKP)).astype(np.float16),
            'wq': np.ascontiguousarray(Wq[:, hc].reshape(DT, 128, 256)).astype(np.float16),
            'wk': np.ascontiguousarray(Wk[:, hc].reshape(DT, 128, 256)).astype(np.float16),
            'wv': np.ascontiguousarray(Wv[:, hc].reshape(DT, 128, 256)).astype(np.float16),
            'wo': np.ascontiguousarray(Wo[hc, :].reshape(2, 128, OUT)),
            'qkb': np.ascontiguousarray(qkb),
            'vb': np.ascontiguousarray(bv[hc].reshape(1, 256)),
            'kbias': np.ascontiguousarray(kbias.reshape(SKT, 128).T),
        })

    res = bass_utils.run_bass_kernel_spmd(
        nc, in_maps, core_ids=list(range(NCORES)), trace=trace)

    out = np.zeros((B, S, OUT), np.float32)
    for b in range(B):
        if host_fallback[b]:
            out[b] = _ref_numpy(query[b], key[b], value[b], Wq, bq, Wk, bk,
                                Wv, bv, Wo, bo,
                                q_mask[b].astype(np.float32),
                                v_mask[b].astype(np.float32))
            continue
        acc = np.zeros((SQP, OUT), np.float32)
        for c in range(4 * b, 4 * b + 4):
            acc += res.results[c]['outp'].astype(np.float32)
        nqb = len(qidx[b])
        out[b][qidx[b]] = acc[:nqb] + bo
    return out, res


def kernel(**inputs):
    out, _ = run(**inputs)
    return out
